# revision 13
# baseline (speedup 1.0000x reference)
"""Trainium2 Bass kernel for MHA with query-axis softmax (nn_MHA_2568390443327).

Reference computation (B=4, N=2048, DIM=1024, 16 heads x 64):
    qkv = x @ w_qkv ; q,k,v = split(qkv)
    scores = (q @ k^T) * scale            # [b,h,i(query),j(key)]
    attn = softmax(scores, axis=QUERY)    # normalized over i, per key j
    y = attn @ v ; out = y @ w_out + b_out

Sharding (8 cores): batch (4) x head-half (2). Each core gets its batch's
x (pre-transposed), the qkv weight columns and w_out rows for its 8 heads,
and produces a partial [DIM, N] fp16 output (transposed). Host sums the two
head-half partials per batch, adds the bias, and transposes back.

v4 design: the exp of all N^2 scores on ScalarE (1 elem/lane/cycle) is the
hard floor, so everything serves ACT saturation while shrinking PE work:
  - Mixed-FD exps: head A of each pair gets one FD=2048 activation from a
    4-bank PSUM slot; head B gets two FD=1024 from a 2-bank slot. Single
    buffering per slot still ping-pongs (A's refill runs under B's exps
    and vice versa). PSUM: 4 (S_A) + 2 (S_B) + 1 (y) + 1 (proj) = 8 banks.
  - Score fills tile-pack 2x using swapped q/k copies (qx/kx hold [B;A]
    so each head has its rows on both partition halves): even i-chunks run
    on PE rows 0-63, odd on 64-127, concurrently.
  - exp outputs (at) are fp8e4: halves SBUF and keeps both y-matmul
    operands fp8 (vp too, scaled by 4096 via the reciprocal path; undone
    in the final output copy). exp runs with bias=-1 as an fp8-overflow
    guard, which cancels exactly in the den normalization.
  - y accumulates over j in 4-j-tile PSUM blocks, emitted one block late,
    one i-chunk per j-step; qkv/out projection chains (256-free) thread
    into remaining PE slack.
"""

import os
import numpy as np

# ---------------------------------------------------------------------------
# Problem constants (hardcoded; kernel.py must be self-contained).
B = 4
N = 2048          # sequence length
F = 1024          # model dim (contraction for qkv proj)
HEADS_TOT = 16
DH = 64           # head dim
HH = 8            # heads per core (head-half)
CH = HH * DH      # 512: per-core hidden
OUT = 1024        # output dim
SCALE = 0.125     # 1/sqrt(64)
N_CORES = 8

P = 128           # partitions
NC512 = 512       # matmul free-dim chunk (one PSUM bank of fp32)
S_W = 1024        # head-B score tile width (half the i range)
JB = 4            # j-tiles per y accumulation block
VPS = 1.0         # vp scale (fp8 experiment disabled)


def _build_nc():
    import concourse.bass as bass  # noqa: F401
    import concourse.mybir as mybir
    from concourse import bacc
    from concourse.tile import TileContext

    f32 = mybir.dt.float32
    f16 = mybir.dt.float16
    f8 = mybir.dt.float8e4
    EXP = mybir.ActivationFunctionType.Exp
    MUL = mybir.AluOpType.mult

    nc = bacc.Bacc(None, target_bir_lowering=False)

    xT = nc.declare_dram_parameter("xT", [F, N], f16, isOutput=False)
    wqkv = nc.declare_dram_parameter("wqkv", [F, 3 * CH], f16, isOutput=False)
    wout = nc.declare_dram_parameter("wout", [CH, OUT], f16, isOutput=False)
    outT = nc.declare_dram_parameter("outT", [OUT, N], f16, isOutput=True)

    KT = F // P            # 8 k-tiles for qkv projection contraction
    NT = N // P            # 16 j-tiles (key blocks)
    PAIRS = 4              # head pairs per core (2 heads each)
    OT = OUT // P          # 8 output row tiles

    with TileContext(nc) as tc:
        with (
            tc.tile_pool(name="p_x", bufs=1) as p_x,
            tc.tile_pool(name="p_w", bufs=1) as p_w,
            tc.tile_pool(name="p_wout", bufs=1) as p_wout,
            tc.tile_pool(name="p_v", bufs=1) as p_v,
            tc.tile_pool(name="p_qkT", bufs=1) as p_qkT,
            tc.tile_pool(name="p_ysb", bufs=1) as p_ysb,
            tc.tile_pool(name="p_atA", bufs=9) as p_atA,
            tc.tile_pool(name="p_atB", bufs=16) as p_atB,
            tc.tile_pool(name="p_den", bufs=64) as p_den,
            tc.tile_pool(name="p_vp", bufs=20) as p_vp,
            tc.tile_pool(name="p_osb", bufs=4) as p_osb,
            tc.tile_pool(name="psSA", bufs=1, space="PSUM") as psSA,
            tc.tile_pool(name="psSB", bufs=1, space="PSUM") as psSB,
            tc.tile_pool(name="psY", bufs=1, space="PSUM") as psY,
            tc.tile_pool(name="psP", bufs=1, space="PSUM") as psP,
        ):
            # ---------------- persistent SBUF tiles ----------------
            xt = [p_x.tile([P, N], f16, tag=f"x{k}", name=f"x{k}")
                  for k in range(KT)]
            wt = [p_w.tile([P, 3 * CH], f16, tag=f"w{k}", name=f"w{k}")
                  for k in range(KT)]
            wout_sb = [p_wout.tile([P, OUT], f16, tag=f"wo{c}", name=f"wo{c}")
                       for c in range(PAIRS)]
            vnat = [p_v.tile([P, CH], f16, tag=f"v{j}", name=f"v{j}")
                    for j in range(NT)]
            # q/k of current+next pair, plus partition-swapped copies [B;A]
            qT = [p_qkT.tile([P, N], f16, tag=f"qT{i}", name=f"qT{i}")
                  for i in range(2)]
            kT = [p_qkT.tile([P, N], f16, tag=f"kT{i}", name=f"kT{i}")
                  for i in range(2)]
            qX = [p_qkT.tile([P, N], f16, tag=f"qX{i}", name=f"qX{i}")
                  for i in range(2)]
            kX = [p_qkT.tile([P, N], f16, tag=f"kX{i}", name=f"kX{i}")
                  for i in range(2)]
            # per-pair fp16 y accumulators (hold 4096*y; all pairs kept)
            y_sb = [p_ysb.tile([P, N], f16, tag=f"y{p_}", name=f"y{p_}")
                    for p_ in range(PAIRS)]

            neg1 = p_osb.tile([P, 1], f32, tag="neg1", name="neg1")
            nc.vector.memset(neg1, -1.0)

            for k in range(KT):
                nc.sync.dma_start(out=wt[k], in_=wqkv[k * P:(k + 1) * P, :])
                nc.sync.dma_start(out=xt[k], in_=xT[k * P:(k + 1) * P, :])
            for c in range(PAIRS):
                nc.sync.dma_start(out=wout_sb[c],
                                  in_=wout[c * P:(c + 1) * P, :])

            # ---------------- projection helpers ----------------
            def emit_v_proj(j, half, width):
                ps = psP.tile([P, width], f32, tag="pp",
                              name=f"pv{j}_{half}")
                c0 = half * width
                for k in range(KT):
                    nc.tensor.matmul(
                        ps,
                        lhsT=xt[k][:, j * P:(j + 1) * P],
                        rhs=wt[k][:, 2 * CH + c0:2 * CH + c0 + width],
                        start=(k == 0), stop=(k == KT - 1))
                nc.vector.tensor_copy(vnat[j][:, c0:c0 + width], ps)

            def emit_qk_chain(pr, sec, ch, width):
                dst = (qT if sec == 0 else kT)[pr % 2]
                ps = psP.tile([P, width], f32, tag="pp",
                              name=f"qk{pr}_{sec}_{ch}")
                i0 = ch * width
                for k in range(KT):
                    nc.tensor.matmul(
                        ps,
                        lhsT=wt[k][:, sec * CH + pr * P:sec * CH + (pr + 1) * P],
                        rhs=xt[k][:, i0:i0 + width],
                        start=(k == 0), stop=(k == KT - 1))
                nc.vector.tensor_copy(dst[:, i0:i0 + width], ps)

            def emit_swap(pr, idx):
                """Build the partition-swapped q/k copies for pair pr:
                qX/kX rows 0-63 = head B, rows 64-127 = head A."""
                b = pr % 2
                src, dst = (qT, qX) if idx // 2 == 0 else (kT, kX)
                if idx % 2 == 0:
                    nc.vector.tensor_copy(dst[b][0:DH, :], src[b][DH:P, :])
                else:
                    nc.vector.tensor_copy(dst[b][DH:P, :], src[b][0:DH, :])

            def emit_out_chain(o, ich):
                ps = psP.tile([P, NC512], f32, tag="pp", name=f"po{o}_{ich}")
                for c in range(PAIRS):
                    nc.tensor.matmul(
                        ps,
                        lhsT=wout_sb[c][:, o * P:(o + 1) * P],
                        rhs=y_sb[c][:, ich * NC512:(ich + 1) * NC512],
                        start=(c == 0), stop=(c == PAIRS - 1))
                osb = p_osb.tile([P, NC512], f16, tag="osb", name="osb")
                nc.vector.tensor_scalar_mul(osb, ps, 1.0 / VPS)
                nc.sync.dma_start(
                    out=outT[o * P:(o + 1) * P,
                             ich * NC512:(ich + 1) * NC512],
                    in_=osb)

            # ---------------- prelude ----------------
            emit_qk_chain(0, 1, 0, 512)
            for ch in range(4):
                emit_qk_chain(0, 0, ch, 512)
            for ch in range(1, 4):
                emit_qk_chain(0, 1, ch, 512)
            for idx in range(4):
                emit_swap(0, idx)
            for j in range(JB):
                emit_v_proj(j, 0, 512)

            def filler_list(pr):
                work = []
                if pr == 0:
                    for j in range(JB, NT):
                        for h in range(2):
                            work.append(("v", j, h))
                if pr + 1 < PAIRS:
                    for ch in range(8):
                        work.append(("k", pr + 1, ch))
                    for ch in range(8):
                        work.append(("q", pr + 1, ch))
                    for idx in range(4):
                        work.append(("s", pr + 1, idx))
                return work

            def emit_filler(item):
                kind, a, b_ = item
                if kind == "v":
                    emit_v_proj(a, b_, 256)
                elif kind == "k":
                    emit_qk_chain(a, 1, b_, 256)
                elif kind == "q":
                    emit_qk_chain(a, 0, b_, 256)
                else:
                    emit_swap(a, b_)

            # ---------------- attention ----------------
            ats = {}    # (pr, ho, j, half) -> at tile (A: half=0 full width)
            vp = {}     # (pr, ho, j) -> fp8 vp tile (v * 4096/den)
            y_queue = []

            def make_y_chunk(pr, blk, ic):
                jlist = list(range(blk * JB, (blk + 1) * JB))

                def emit():
                    hf, c2 = divmod(ic, 2)
                    yb = psY.tile([P, NC512], f32, tag="yy",
                                  name=f"yb{pr}_{blk}_{ic}")
                    for idx, jj in enumerate(jlist):
                        for ho in (0, DH):
                            if ho == 0:
                                rhs = ats[(pr, 0, jj, 0)][:, ic * NC512:
                                                          (ic + 1) * NC512]
                            else:
                                rhs = ats[(pr, DH, jj, hf)][:, c2 * NC512:
                                                            (c2 + 1) * NC512]
                            nc.tensor.matmul(
                                yb[ho:ho + DH, :],
                                lhsT=vp[(pr, ho, jj)],
                                rhs=rhs,
                                start=(idx == 0),
                                stop=(idx == JB - 1),
                                tile_position=(0, ho))
                    i0 = hf * S_W + c2 * NC512
                    if blk == 0:
                        nc.vector.tensor_copy(y_sb[pr][:, i0:i0 + NC512], yb)
                    else:
                        nc.vector.tensor_add(
                            y_sb[pr][:, i0:i0 + NC512],
                            y_sb[pr][:, i0:i0 + NC512], yb)
                    if ic == 3:
                        for jj in jlist:
                            ats.pop((pr, 0, jj, 0), None)
                            ats.pop((pr, DH, jj, 0), None)
                            ats.pop((pr, DH, jj, 1), None)
                            vp.pop((pr, 0, jj), None)
                            vp.pop((pr, DH, jj), None)
                return emit

            def fill_scores(pr, j, ho, s, width, ioff):
                """Packed score fill: even 512-chunks on PE rows 0-63, odd
                chunks on rows 64-127 via the swapped copies."""
                b = pr % 2
                js = slice(j * P, (j + 1) * P)
                nch = width // NC512
                for c in range(nch):
                    i0 = ioff + c * NC512
                    if (c % 2 == 0) == (ho == 0):
                        # head's rows live at partitions 0-63 in this source
                        kt_, qt_ = (kT[b], qT[b]) if ho == 0 else (kX[b], qX[b])
                        nc.tensor.matmul(
                            s[:, c * NC512:(c + 1) * NC512],
                            lhsT=kt_[0:DH, js],
                            rhs=qt_[0:DH, i0:i0 + NC512],
                            start=True, stop=True,
                            tile_position=(0, 0))
                    else:
                        kt_, qt_ = (kX[b], qX[b]) if ho == 0 else (kT[b], qT[b])
                        nc.tensor.matmul(
                            s[:, c * NC512:(c + 1) * NC512],
                            lhsT=kt_[DH:P, js],
                            rhs=qt_[DH:P, i0:i0 + NC512],
                            start=True, stop=True,
                            tile_position=(DH, 0))

            for pr in range(PAIRS):
                filler = filler_list(pr)
                fi = 0
                for j in range(NT):
                    quota = -(-(len(filler) - fi) // (NT - j))  # ceil
                    if fi < len(filler) and quota > 0:
                        emit_filler(filler[fi])
                        fi += 1
                        quota -= 1
                    dens = {}
                    # head A: one FD=2048 exp from the 4-bank slot
                    sA = psSA.tile([P, N], f32, tag="sa", name=f"sa{pr}_{j}")
                    fill_scores(pr, j, 0, sA, N, 0)
                    atA = p_atA.tile([P, N], f16, tag="atA", name="atA")
                    denA = p_den.tile([P, 1], f32, tag="den", name="denA")
                    nc.scalar.activation(atA, sA, EXP, bias=neg1,
                                         scale=SCALE, accum_out=denA)
                    ats[(pr, 0, j, 0)] = atA
                    # pending y-chunk between the A and B groups
                    if y_queue:
                        y_queue.pop(0)()
                    # head B: two FD=1024 exps from the 2-bank slot
                    for hf in range(2):
                        sB = psSB.tile([P, S_W], f32, tag="sb",
                                       name=f"sb{pr}_{j}_{hf}")
                        fill_scores(pr, j, DH, sB, S_W, hf * S_W)
                        atB = p_atB.tile([P, S_W], f16, tag="atB", name="atB")
                        den = p_den.tile([P, 1], f32, tag="den",
                                         name=f"denB{hf}")
                        nc.scalar.activation(atB, sB, EXP, bias=neg1,
                                             scale=SCALE, accum_out=den)
                        ats[(pr, DH, j, hf)] = atB
                        dens[hf] = den
                    # rest of this step's filler quota
                    while fi < len(filler) and quota > 0:
                        emit_filler(filler[fi])
                        fi += 1
                        quota -= 1
                    # denominators -> reciprocal -> fp8 vp (scaled by VPS)
                    recA = p_den.tile([P, 1], f32, tag="den", name="recA")
                    nc.vector.reciprocal(recA, denA)
                    vpA = p_vp.tile([P, DH], f16, tag="vp", name="vpA")
                    nc.vector.tensor_scalar(
                        vpA, vnat[j][:, pr * 2 * DH:pr * 2 * DH + DH],
                        recA, VPS, MUL, MUL)
                    vp[(pr, 0, j)] = vpA
                    dtot = p_den.tile([P, 1], f32, tag="den", name="dtot")
                    nc.vector.tensor_add(dtot, dens[0], dens[1])
                    recB = p_den.tile([P, 1], f32, tag="den", name="recB")
                    nc.vector.reciprocal(recB, dtot)
                    vpB = p_vp.tile([P, DH], f16, tag="vp", name="vpB")
                    c0 = pr * 2 * DH + DH
                    nc.vector.tensor_scalar(
                        vpB, vnat[j][:, c0:c0 + DH], recB, VPS, MUL, MUL)
                    vp[(pr, DH, j)] = vpB
                    if j % JB == JB - 1:
                        blk = j // JB
                        for ic in range(4):
                            y_queue.append(make_y_chunk(pr, blk, ic))
                while fi < len(filler):
                    emit_filler(filler[fi])
                    fi += 1

            # drain remaining y chunks (last block of last pair)
            while y_queue:
                y_queue.pop(0)()

            # ---------------- output projection (tail) ----------------
            for o in range(OT):
                for ich in range(N // NC512):
                    emit_out_chain(o, ich)
    return nc


def _shard_inputs(x, w_qkv, w_out):
    """Build per-core input maps: core c -> (batch c//2, head-half c%2)."""
    in_maps = []
    for c in range(N_CORES):
        b, hh = c // 2, c % 2
        cols = slice(hh * CH, (hh + 1) * CH)
        xTc = np.ascontiguousarray(np.asarray(x[b]).T, dtype=np.float16)
        wq = w_qkv[:, 0 * F:1 * F][:, cols]
        wk = w_qkv[:, 1 * F:2 * F][:, cols]
        wv = w_qkv[:, 2 * F:3 * F][:, cols]
        wqkv_c = np.ascontiguousarray(
            np.concatenate([wq, wk, wv], axis=1), dtype=np.float16)
        wout_c = np.ascontiguousarray(w_out[cols, :], dtype=np.float16)
        in_maps.append({"xT": xTc, "wqkv": wqkv_c, "wout": wout_c})
    return in_maps


def _gather_outputs(results, b_out):
    out = np.empty((B, N, OUT), np.float32)
    bias = np.asarray(b_out, dtype=np.float32)
    for b in range(B):
        acc = (results[2 * b]["outT"].astype(np.float32)
               + results[2 * b + 1]["outT"].astype(np.float32))  # [OUT, N]
        out[b] = acc.T + bias
    return out


# Test instrumentation (harness just calls kernel(); these stay default).
_TRACE = False
_LAST_RESULT = None


def kernel(x, w_qkv, w_out, b_out):
    global _LAST_RESULT
    # The bass->PJRT path needs the axon trn2 devices visible to jax.
    if os.environ.get("JAX_PLATFORMS") not in (None, "", "axon"):
        os.environ.pop("JAX_PLATFORMS", None)
    from concourse.bass_utils import run_bass_kernel_spmd

    nc = _build_nc()
    if not nc.is_finalized():
        nc.finalize()  # runs Bacc legalization (wait splitting, reg alloc)
    in_maps = _shard_inputs(np.asarray(x), np.asarray(w_qkv),
                            np.asarray(w_out))
    res = run_bass_kernel_spmd(nc, in_maps, list(range(N_CORES)),
                               trace=_TRACE)
    _LAST_RESULT = res
    return _gather_outputs(res.results, np.asarray(b_out))


# ---------------------------------------------------------------------------
# Numpy emulation of the per-core device program (for host-logic testing;
# not used by kernel()).
def _emulate_core(m):
    xT, wqkv, wout = m["xT"], m["wqkv"], m["wout"]
    qT = (wqkv[:, 0:CH].T @ xT)          # [CH, N]
    kTm = (wqkv[:, CH:2 * CH].T @ xT)    # [CH, N]
    v = xT.T @ wqkv[:, 2 * CH:3 * CH]    # [N, CH]
    y = np.empty((CH, N), np.float32)
    for h in range(HH):
        qh = qT[h * DH:(h + 1) * DH, :]      # [DH, N(i)]
        kh = kTm[h * DH:(h + 1) * DH, :]     # [DH, N(j)]
        sT = kh.T @ qh                       # [j, i]
        e = np.exp(sT * SCALE - 1.0)
        den = e.sum(axis=1, keepdims=True)   # over queries i, per key j
        vpm = v[:, h * DH:(h + 1) * DH] * (VPS / den)
        y[h * DH:(h + 1) * DH, :] = (vpm.T @ e) / VPS  # [DH, i]
    outT_acc = wout.T @ y                    # [OUT, N]
    return outT_acc.astype(np.float16)


def _kernel_emulated(x, w_qkv, w_out, b_out):
    in_maps = _shard_inputs(np.asarray(x), np.asarray(w_qkv),
                            np.asarray(w_out))
    results = [{"outT": _emulate_core(m)} for m in in_maps]
    return _gather_outputs(results, np.asarray(b_out))


# revision 14
# speedup vs baseline: 1.0494x; 1.0494x over previous
"""Trainium2 Bass kernel for MHA with query-axis softmax (nn_MHA_2568390443327).

Reference computation (B=4, N=2048, DIM=1024, 16 heads x 64):
    qkv = x @ w_qkv ; q,k,v = split(qkv)
    scores = (q @ k^T) * scale            # [b,h,i(query),j(key)]
    attn = softmax(scores, axis=QUERY)    # normalized over i, per key j
    y = attn @ v ; out = y @ w_out + b_out

Sharding (8 cores): batch (4) x head-half (2). Each core gets its batch's
x (pre-transposed), the qkv weight columns and w_out rows for its 8 heads,
and produces a partial [DIM, N] fp16 output (transposed). Host sums the two
head-half partials per batch, adds the bias, and transposes back.

v4 design: the exp of all N^2 scores on ScalarE (1 elem/lane/cycle) is the
hard floor, so everything serves ACT saturation while shrinking PE work:
  - Mixed-FD exps: head A of each pair gets one FD=2048 activation from a
    4-bank PSUM slot; head B gets two FD=1024 from a 2-bank slot. Single
    buffering per slot still ping-pongs (A's refill runs under B's exps
    and vice versa). PSUM: 4 (S_A) + 2 (S_B) + 1 (y) + 1 (proj) = 8 banks.
  - Score fills tile-pack 2x using swapped q/k copies (qx/kx hold [B;A]
    so each head has its rows on both partition halves): even i-chunks run
    on PE rows 0-63, odd on 64-127, concurrently.
  - exp outputs (at) are fp8e4: halves SBUF and keeps both y-matmul
    operands fp8 (vp too, scaled by 4096 via the reciprocal path; undone
    in the final output copy). exp runs with bias=-1 as an fp8-overflow
    guard, which cancels exactly in the den normalization.
  - y accumulates over j in 4-j-tile PSUM blocks, emitted one block late,
    one i-chunk per j-step; qkv/out projection chains (256-free) thread
    into remaining PE slack.
"""

import os
import numpy as np

# ---------------------------------------------------------------------------
# Problem constants (hardcoded; kernel.py must be self-contained).
B = 4
N = 2048          # sequence length
F = 1024          # model dim (contraction for qkv proj)
HEADS_TOT = 16
DH = 64           # head dim
HH = 8            # heads per core (head-half)
CH = HH * DH      # 512: per-core hidden
OUT = 1024        # output dim
SCALE = 0.125     # 1/sqrt(64)
N_CORES = 8

P = 128           # partitions
NC512 = 512       # matmul free-dim chunk (one PSUM bank of fp32)
S_W = 1024        # head-B score tile width (half the i range)
JB = 4            # j-tiles per y accumulation block
VPS = 1.0         # vp scale (fp8 experiment disabled)


def _build_nc():
    import concourse.bass as bass  # noqa: F401
    import concourse.mybir as mybir
    from concourse import bacc
    from concourse.tile import TileContext

    f32 = mybir.dt.float32
    f16 = mybir.dt.float16
    f8 = mybir.dt.float8e4
    EXP = mybir.ActivationFunctionType.Exp
    MUL = mybir.AluOpType.mult

    nc = bacc.Bacc(None, target_bir_lowering=False)

    xT = nc.declare_dram_parameter("xT", [F, N], f16, isOutput=False)
    wqkv = nc.declare_dram_parameter("wqkv", [F, 3 * CH], f16, isOutput=False)
    wout = nc.declare_dram_parameter("wout", [CH, OUT], f16, isOutput=False)
    outT = nc.declare_dram_parameter("outT", [OUT, N], f16, isOutput=True)

    KT = F // P            # 8 k-tiles for qkv projection contraction
    NT = N // P            # 16 j-tiles (key blocks)
    PAIRS = 4              # head pairs per core (2 heads each)
    OT = OUT // P          # 8 output row tiles

    with TileContext(nc) as tc:
        with (
            tc.tile_pool(name="p_x", bufs=1) as p_x,
            tc.tile_pool(name="p_w", bufs=1) as p_w,
            tc.tile_pool(name="p_wout", bufs=1) as p_wout,
            tc.tile_pool(name="p_v", bufs=1) as p_v,
            tc.tile_pool(name="p_qkT", bufs=1) as p_qkT,
            tc.tile_pool(name="p_ysb", bufs=1) as p_ysb,
            tc.tile_pool(name="p_atA", bufs=9) as p_atA,
            tc.tile_pool(name="p_atB", bufs=16) as p_atB,
            tc.tile_pool(name="p_den", bufs=64) as p_den,
            tc.tile_pool(name="p_vp", bufs=20) as p_vp,
            tc.tile_pool(name="p_osb", bufs=4) as p_osb,
            tc.tile_pool(name="psSA", bufs=1, space="PSUM") as psSA,
            tc.tile_pool(name="psSB", bufs=1, space="PSUM") as psSB,
            tc.tile_pool(name="psY", bufs=1, space="PSUM") as psY,
            tc.tile_pool(name="psP", bufs=1, space="PSUM") as psP,
        ):
            # ---------------- persistent SBUF tiles ----------------
            xt = [p_x.tile([P, N], f16, tag=f"x{k}", name=f"x{k}")
                  for k in range(KT)]
            wt = [p_w.tile([P, 3 * CH], f16, tag=f"w{k}", name=f"w{k}")
                  for k in range(KT)]
            wout_sb = [p_wout.tile([P, OUT], f16, tag=f"wo{c}", name=f"wo{c}")
                       for c in range(PAIRS)]
            vnat = [p_v.tile([P, CH], f16, tag=f"v{j}", name=f"v{j}")
                    for j in range(NT)]
            # q/k of current+next pair, plus partition-swapped copies [B;A]
            qT = [p_qkT.tile([P, N], f16, tag=f"qT{i}", name=f"qT{i}")
                  for i in range(2)]
            kT = [p_qkT.tile([P, N], f16, tag=f"kT{i}", name=f"kT{i}")
                  for i in range(2)]
            qX = [p_qkT.tile([P, N], f16, tag=f"qX{i}", name=f"qX{i}")
                  for i in range(2)]
            kX = [p_qkT.tile([P, N], f16, tag=f"kX{i}", name=f"kX{i}")
                  for i in range(2)]
            # per-pair fp16 y accumulators (hold 4096*y; all pairs kept)
            y_sb = [p_ysb.tile([P, N], f16, tag=f"y{p_}", name=f"y{p_}")
                    for p_ in range(PAIRS)]

            neg1 = p_osb.tile([P, 1], f32, tag="neg1", name="neg1")
            nc.vector.memset(neg1, -1.0)

            for k in range(KT):
                nc.sync.dma_start(out=wt[k], in_=wqkv[k * P:(k + 1) * P, :])
                nc.sync.dma_start(out=xt[k], in_=xT[k * P:(k + 1) * P, :])
            for c in range(PAIRS):
                nc.sync.dma_start(out=wout_sb[c],
                                  in_=wout[c * P:(c + 1) * P, :])

            # ---------------- projection helpers ----------------
            def emit_v_proj(j, half, width):
                ps = psP.tile([P, width], f32, tag="pp",
                              name=f"pv{j}_{half}")
                c0 = half * width
                for k in range(KT):
                    nc.tensor.matmul(
                        ps,
                        lhsT=xt[k][:, j * P:(j + 1) * P],
                        rhs=wt[k][:, 2 * CH + c0:2 * CH + c0 + width],
                        start=(k == 0), stop=(k == KT - 1))
                nc.vector.tensor_copy(vnat[j][:, c0:c0 + width], ps)

            def emit_qk_chain(pr, sec, ch, width):
                dst = (qT if sec == 0 else kT)[pr % 2]
                ps = psP.tile([P, width], f32, tag="pp",
                              name=f"qk{pr}_{sec}_{ch}")
                i0 = ch * width
                for k in range(KT):
                    nc.tensor.matmul(
                        ps,
                        lhsT=wt[k][:, sec * CH + pr * P:sec * CH + (pr + 1) * P],
                        rhs=xt[k][:, i0:i0 + width],
                        start=(k == 0), stop=(k == KT - 1))
                nc.vector.tensor_copy(dst[:, i0:i0 + width], ps)

            def emit_swap(pr, idx):
                """Build the partition-swapped q/k copies for pair pr:
                qX/kX rows 0-63 = head B, rows 64-127 = head A."""
                b = pr % 2
                src, dst = (qT, qX) if idx // 2 == 0 else (kT, kX)
                if idx % 2 == 0:
                    nc.vector.tensor_copy(dst[b][0:DH, :], src[b][DH:P, :])
                else:
                    nc.vector.tensor_copy(dst[b][DH:P, :], src[b][0:DH, :])

            def emit_out_chain(o, ich):
                pool = psP if (o * 4 + ich) % 2 == 0 else psY
                tag = "pp" if pool is psP else "yy"
                ps = pool.tile([P, NC512], f32, tag=tag, name=f"po{o}_{ich}")
                for c in range(PAIRS):
                    nc.tensor.matmul(
                        ps,
                        lhsT=wout_sb[c][:, o * P:(o + 1) * P],
                        rhs=y_sb[c][:, ich * NC512:(ich + 1) * NC512],
                        start=(c == 0), stop=(c == PAIRS - 1))
                osb = p_osb.tile([P, NC512], f16, tag="osb", name="osb")
                nc.vector.tensor_scalar_mul(osb, ps, 1.0 / VPS)
                nc.sync.dma_start(
                    out=outT[o * P:(o + 1) * P,
                             ich * NC512:(ich + 1) * NC512],
                    in_=osb)

            # ---------------- prelude ----------------
            emit_qk_chain(0, 1, 0, 512)
            for ch in range(4):
                emit_qk_chain(0, 0, ch, 512)
            for ch in range(1, 4):
                emit_qk_chain(0, 1, ch, 512)
            for idx in range(4):
                emit_swap(0, idx)
            for j in range(JB):
                emit_v_proj(j, 0, 512)

            def filler_list(pr):
                work = []
                if pr == 0:
                    for j in range(JB, NT):
                        for h in range(2):
                            work.append(("v", j, h))
                if pr + 1 < PAIRS:
                    for ch in range(8):
                        work.append(("k", pr + 1, ch))
                    for ch in range(8):
                        work.append(("q", pr + 1, ch))
                    for idx in range(4):
                        work.append(("s", pr + 1, idx))
                return work

            def emit_filler(item):
                kind, a, b_ = item
                if kind == "v":
                    emit_v_proj(a, b_, 256)
                elif kind == "k":
                    emit_qk_chain(a, 1, b_, 256)
                elif kind == "q":
                    emit_qk_chain(a, 0, b_, 256)
                else:
                    emit_swap(a, b_)

            # ---------------- attention ----------------
            ats = {}    # (pr, ho, j, half) -> at tile (A: half=0 full width)
            vp = {}     # (pr, ho, j) -> fp8 vp tile (v * 4096/den)
            y_queue = []

            def make_y_chunk(pr, blk, ic):
                jlist = list(range(blk * JB, (blk + 1) * JB))

                def emit():
                    hf, c2 = divmod(ic, 2)
                    yb = psY.tile([P, NC512], f32, tag="yy",
                                  name=f"yb{pr}_{blk}_{ic}")
                    for idx, jj in enumerate(jlist):
                        for ho in (0, DH):
                            if ho == 0:
                                rhs = ats[(pr, 0, jj, 0)][:, ic * NC512:
                                                          (ic + 1) * NC512]
                            else:
                                rhs = ats[(pr, DH, jj, hf)][:, c2 * NC512:
                                                            (c2 + 1) * NC512]
                            nc.tensor.matmul(
                                yb[ho:ho + DH, :],
                                lhsT=vp[(pr, ho, jj)],
                                rhs=rhs,
                                start=(idx == 0),
                                stop=(idx == JB - 1),
                                tile_position=(0, ho))
                    i0 = hf * S_W + c2 * NC512
                    if blk == 0:
                        nc.vector.tensor_copy(y_sb[pr][:, i0:i0 + NC512], yb)
                    else:
                        nc.vector.tensor_add(
                            y_sb[pr][:, i0:i0 + NC512],
                            y_sb[pr][:, i0:i0 + NC512], yb)
                    if ic == 3:
                        for jj in jlist:
                            ats.pop((pr, 0, jj, 0), None)
                            ats.pop((pr, DH, jj, 0), None)
                            ats.pop((pr, DH, jj, 1), None)
                            vp.pop((pr, 0, jj), None)
                            vp.pop((pr, DH, jj), None)
                return emit

            def fill_scores(pr, j, ho, s, width, ioff):
                """Packed score fill: even 512-chunks on PE rows 0-63, odd
                chunks on rows 64-127 via the swapped copies."""
                b = pr % 2
                js = slice(j * P, (j + 1) * P)
                nch = width // NC512
                for c in range(nch):
                    i0 = ioff + c * NC512
                    if (c % 2 == 0) == (ho == 0):
                        # head's rows live at partitions 0-63 in this source
                        kt_, qt_ = (kT[b], qT[b]) if ho == 0 else (kX[b], qX[b])
                        nc.tensor.matmul(
                            s[:, c * NC512:(c + 1) * NC512],
                            lhsT=kt_[0:DH, js],
                            rhs=qt_[0:DH, i0:i0 + NC512],
                            start=True, stop=True,
                            tile_position=(0, 0))
                    else:
                        kt_, qt_ = (kX[b], qX[b]) if ho == 0 else (kT[b], qT[b])
                        nc.tensor.matmul(
                            s[:, c * NC512:(c + 1) * NC512],
                            lhsT=kt_[DH:P, js],
                            rhs=qt_[DH:P, i0:i0 + NC512],
                            start=True, stop=True,
                            tile_position=(DH, 0))

            for pr in range(PAIRS):
                filler = filler_list(pr)
                fi = 0
                for j in range(NT):
                    quota = -(-(len(filler) - fi) // (NT - j))  # ceil
                    if fi < len(filler) and quota > 0:
                        emit_filler(filler[fi])
                        fi += 1
                        quota -= 1
                    dens = {}
                    # head A: one FD=2048 exp from the 4-bank slot
                    sA = psSA.tile([P, N], f32, tag="sa", name=f"sa{pr}_{j}")
                    fill_scores(pr, j, 0, sA, N, 0)
                    atA = p_atA.tile([P, N], f16, tag="atA", name="atA")
                    denA = p_den.tile([P, 1], f32, tag="den", name="denA")
                    nc.scalar.activation(atA, sA, EXP, bias=neg1,
                                         scale=SCALE, accum_out=denA)
                    ats[(pr, 0, j, 0)] = atA
                    # pending y-chunk between the A and B groups
                    if y_queue:
                        y_queue.pop(0)()
                    # head B: two FD=1024 exps from the 2-bank slot
                    for hf in range(2):
                        sB = psSB.tile([P, S_W], f32, tag="sb",
                                       name=f"sb{pr}_{j}_{hf}")
                        fill_scores(pr, j, DH, sB, S_W, hf * S_W)
                        atB = p_atB.tile([P, S_W], f16, tag="atB", name="atB")
                        den = p_den.tile([P, 1], f32, tag="den",
                                         name=f"denB{hf}")
                        nc.scalar.activation(atB, sB, EXP, bias=neg1,
                                             scale=SCALE, accum_out=den)
                        ats[(pr, DH, j, hf)] = atB
                        dens[hf] = den
                    # rest of this step's filler quota
                    while fi < len(filler) and quota > 0:
                        emit_filler(filler[fi])
                        fi += 1
                        quota -= 1
                    # denominators -> reciprocal -> fp8 vp (scaled by VPS)
                    recA = p_den.tile([P, 1], f32, tag="den", name="recA")
                    nc.vector.reciprocal(recA, denA)
                    vpA = p_vp.tile([P, DH], f16, tag="vp", name="vpA")
                    nc.vector.tensor_scalar(
                        vpA, vnat[j][:, pr * 2 * DH:pr * 2 * DH + DH],
                        recA, VPS, MUL, MUL)
                    vp[(pr, 0, j)] = vpA
                    dtot = p_den.tile([P, 1], f32, tag="den", name="dtot")
                    nc.vector.tensor_add(dtot, dens[0], dens[1])
                    recB = p_den.tile([P, 1], f32, tag="den", name="recB")
                    nc.vector.reciprocal(recB, dtot)
                    vpB = p_vp.tile([P, DH], f16, tag="vp", name="vpB")
                    c0 = pr * 2 * DH + DH
                    nc.vector.tensor_scalar(
                        vpB, vnat[j][:, c0:c0 + DH], recB, VPS, MUL, MUL)
                    vp[(pr, DH, j)] = vpB
                    if j % JB == JB - 1:
                        blk = j // JB
                        for ic in range(4):
                            y_queue.append(make_y_chunk(pr, blk, ic))
                while fi < len(filler):
                    emit_filler(filler[fi])
                    fi += 1

            # drain remaining y chunks (last block of last pair)
            while y_queue:
                y_queue.pop(0)()

            # ---------------- output projection (tail) ----------------
            for o in range(OT):
                for ich in range(N // NC512):
                    emit_out_chain(o, ich)
    return nc


def _shard_inputs(x, w_qkv, w_out):
    """Build per-core input maps: core c -> (batch c//2, head-half c%2)."""
    in_maps = []
    for c in range(N_CORES):
        b, hh = c // 2, c % 2
        cols = slice(hh * CH, (hh + 1) * CH)
        xTc = np.ascontiguousarray(np.asarray(x[b]).T, dtype=np.float16)
        wq = w_qkv[:, 0 * F:1 * F][:, cols]
        wk = w_qkv[:, 1 * F:2 * F][:, cols]
        wv = w_qkv[:, 2 * F:3 * F][:, cols]
        wqkv_c = np.ascontiguousarray(
            np.concatenate([wq, wk, wv], axis=1), dtype=np.float16)
        wout_c = np.ascontiguousarray(w_out[cols, :], dtype=np.float16)
        in_maps.append({"xT": xTc, "wqkv": wqkv_c, "wout": wout_c})
    return in_maps


def _gather_outputs(results, b_out):
    out = np.empty((B, N, OUT), np.float32)
    bias = np.asarray(b_out, dtype=np.float32)
    for b in range(B):
        acc = (results[2 * b]["outT"].astype(np.float32)
               + results[2 * b + 1]["outT"].astype(np.float32))  # [OUT, N]
        out[b] = acc.T + bias
    return out


# Test instrumentation (harness just calls kernel(); these stay default).
_TRACE = False
_LAST_RESULT = None


def kernel(x, w_qkv, w_out, b_out):
    global _LAST_RESULT
    # The bass->PJRT path needs the axon trn2 devices visible to jax.
    if os.environ.get("JAX_PLATFORMS") not in (None, "", "axon"):
        os.environ.pop("JAX_PLATFORMS", None)
    from concourse.bass_utils import run_bass_kernel_spmd

    nc = _build_nc()
    if not nc.is_finalized():
        nc.finalize()  # runs Bacc legalization (wait splitting, reg alloc)
    in_maps = _shard_inputs(np.asarray(x), np.asarray(w_qkv),
                            np.asarray(w_out))
    res = run_bass_kernel_spmd(nc, in_maps, list(range(N_CORES)),
                               trace=_TRACE)
    _LAST_RESULT = res
    return _gather_outputs(res.results, np.asarray(b_out))


# ---------------------------------------------------------------------------
# Numpy emulation of the per-core device program (for host-logic testing;
# not used by kernel()).
def _emulate_core(m):
    xT, wqkv, wout = m["xT"], m["wqkv"], m["wout"]
    qT = (wqkv[:, 0:CH].T @ xT)          # [CH, N]
    kTm = (wqkv[:, CH:2 * CH].T @ xT)    # [CH, N]
    v = xT.T @ wqkv[:, 2 * CH:3 * CH]    # [N, CH]
    y = np.empty((CH, N), np.float32)
    for h in range(HH):
        qh = qT[h * DH:(h + 1) * DH, :]      # [DH, N(i)]
        kh = kTm[h * DH:(h + 1) * DH, :]     # [DH, N(j)]
        sT = kh.T @ qh                       # [j, i]
        e = np.exp(sT * SCALE - 1.0)
        den = e.sum(axis=1, keepdims=True)   # over queries i, per key j
        vpm = v[:, h * DH:(h + 1) * DH] * (VPS / den)
        y[h * DH:(h + 1) * DH, :] = (vpm.T @ e) / VPS  # [DH, i]
    outT_acc = wout.T @ y                    # [OUT, N]
    return outT_acc.astype(np.float16)


def _kernel_emulated(x, w_qkv, w_out, b_out):
    in_maps = _shard_inputs(np.asarray(x), np.asarray(w_qkv),
                            np.asarray(w_out))
    results = [{"outT": _emulate_core(m)} for m in in_maps]
    return _gather_outputs(results, np.asarray(b_out))


# revision 15
# speedup vs baseline: 1.0591x; 1.0093x over previous
"""Trainium2 Bass kernel for MHA with query-axis softmax (nn_MHA_2568390443327).

Reference computation (B=4, N=2048, DIM=1024, 16 heads x 64):
    qkv = x @ w_qkv ; q,k,v = split(qkv)
    scores = (q @ k^T) * scale            # [b,h,i(query),j(key)]
    attn = softmax(scores, axis=QUERY)    # normalized over i, per key j
    y = attn @ v ; out = y @ w_out + b_out

Sharding (8 cores): batch (4) x head-half (2). Each core gets its batch's
x (pre-transposed), the qkv weight columns and w_out rows for its 8 heads,
and produces a partial [DIM, N] fp16 output (transposed). Host sums the two
head-half partials per batch, adds the bias, and transposes back.

v4 design: the exp of all N^2 scores on ScalarE (1 elem/lane/cycle) is the
hard floor, so everything serves ACT saturation while shrinking PE work:
  - Mixed-FD exps: head A of each pair gets one FD=2048 activation from a
    4-bank PSUM slot; head B gets two FD=1024 from a 2-bank slot. Single
    buffering per slot still ping-pongs (A's refill runs under B's exps
    and vice versa). PSUM: 4 (S_A) + 2 (S_B) + 1 (y) + 1 (proj) = 8 banks.
  - Score fills tile-pack 2x using swapped q/k copies (qx/kx hold [B;A]
    so each head has its rows on both partition halves): even i-chunks run
    on PE rows 0-63, odd on 64-127, concurrently.
  - exp outputs (at) are fp8e4: halves SBUF and keeps both y-matmul
    operands fp8 (vp too, scaled by 4096 via the reciprocal path; undone
    in the final output copy). exp runs with bias=-1 as an fp8-overflow
    guard, which cancels exactly in the den normalization.
  - y accumulates over j in 4-j-tile PSUM blocks, emitted one block late,
    one i-chunk per j-step; qkv/out projection chains (256-free) thread
    into remaining PE slack.
"""

import os
import numpy as np

# ---------------------------------------------------------------------------
# Problem constants (hardcoded; kernel.py must be self-contained).
B = 4
N = 2048          # sequence length
F = 1024          # model dim (contraction for qkv proj)
HEADS_TOT = 16
DH = 64           # head dim
HH = 8            # heads per core (head-half)
CH = HH * DH      # 512: per-core hidden
OUT = 1024        # output dim
SCALE = 0.125     # 1/sqrt(64)
N_CORES = 8

P = 128           # partitions
NC512 = 512       # matmul free-dim chunk (one PSUM bank of fp32)
S_W = 1024        # head-B score tile width (half the i range)
JB = 4            # j-tiles per y accumulation block
VPS = 1.0         # vp scale (fp8 experiment disabled)


def _build_nc():
    import concourse.bass as bass  # noqa: F401
    import concourse.mybir as mybir
    from concourse import bacc
    from concourse.tile import TileContext

    f32 = mybir.dt.float32
    f16 = mybir.dt.float16
    f8 = mybir.dt.float8e4
    EXP = mybir.ActivationFunctionType.Exp
    MUL = mybir.AluOpType.mult

    nc = bacc.Bacc(None, target_bir_lowering=False)

    xT = nc.declare_dram_parameter("xT", [F, N], f16, isOutput=False)
    wqkv = nc.declare_dram_parameter("wqkv", [F, 3 * CH], f16, isOutput=False)
    wout = nc.declare_dram_parameter("wout", [CH, OUT], f16, isOutput=False)
    outT = nc.declare_dram_parameter("outT", [OUT, N], f16, isOutput=True)

    KT = F // P            # 8 k-tiles for qkv projection contraction
    NT = N // P            # 16 j-tiles (key blocks)
    PAIRS = 4              # head pairs per core (2 heads each)
    OT = OUT // P          # 8 output row tiles

    with TileContext(nc) as tc:
        with (
            tc.tile_pool(name="p_x", bufs=1) as p_x,
            tc.tile_pool(name="p_w", bufs=1) as p_w,
            tc.tile_pool(name="p_wout", bufs=1) as p_wout,
            tc.tile_pool(name="p_v", bufs=1) as p_v,
            tc.tile_pool(name="p_qkT", bufs=1) as p_qkT,
            tc.tile_pool(name="p_ysb", bufs=1) as p_ysb,
            tc.tile_pool(name="p_atA", bufs=9) as p_atA,
            tc.tile_pool(name="p_atB", bufs=16) as p_atB,
            tc.tile_pool(name="p_den", bufs=64) as p_den,
            tc.tile_pool(name="p_vp", bufs=20) as p_vp,
            tc.tile_pool(name="p_osb", bufs=4) as p_osb,
            tc.tile_pool(name="psSA", bufs=1, space="PSUM") as psSA,
            tc.tile_pool(name="psSB", bufs=1, space="PSUM") as psSB,
            tc.tile_pool(name="psY", bufs=1, space="PSUM") as psY,
            tc.tile_pool(name="psP", bufs=1, space="PSUM") as psP,
        ):
            # ---------------- persistent SBUF tiles ----------------
            xt = [p_x.tile([P, N], f16, tag=f"x{k}", name=f"x{k}")
                  for k in range(KT)]
            wt = [p_w.tile([P, 3 * CH], f16, tag=f"w{k}", name=f"w{k}")
                  for k in range(KT)]
            wout_sb = [p_wout.tile([P, OUT], f16, tag=f"wo{c}", name=f"wo{c}")
                       for c in range(PAIRS)]
            vnat = [p_v.tile([P, CH], f16, tag=f"v{j}", name=f"v{j}")
                    for j in range(NT)]
            # q/k of current+next pair, plus partition-swapped copies [B;A]
            qT = [p_qkT.tile([P, N], f16, tag=f"qT{i}", name=f"qT{i}")
                  for i in range(2)]
            kT = [p_qkT.tile([P, N], f16, tag=f"kT{i}", name=f"kT{i}")
                  for i in range(2)]
            qX = [p_qkT.tile([P, N], f16, tag=f"qX{i}", name=f"qX{i}")
                  for i in range(2)]
            kX = [p_qkT.tile([P, N], f16, tag=f"kX{i}", name=f"kX{i}")
                  for i in range(2)]
            # per-pair fp16 y accumulators (hold 4096*y; all pairs kept)
            y_sb = [p_ysb.tile([P, N], f16, tag=f"y{p_}", name=f"y{p_}")
                    for p_ in range(PAIRS)]

            neg1 = p_osb.tile([P, 1], f32, tag="neg1", name="neg1")
            nc.vector.memset(neg1, -1.0)

            # DMA in dependency order: pair-0 k/q weight sections and the
            # first x i-chunk land first so projection chains start early.
            for k in range(KT):
                nc.sync.dma_start(out=wt[k][:, CH:2 * CH],
                                  in_=wqkv[k * P:(k + 1) * P, CH:2 * CH])
            for k in range(KT):
                nc.sync.dma_start(out=xt[k][:, 0:NC512],
                                  in_=xT[k * P:(k + 1) * P, 0:NC512])
            for k in range(KT):
                nc.sync.dma_start(out=wt[k][:, 0:CH],
                                  in_=wqkv[k * P:(k + 1) * P, 0:CH])
            for ch in (1, 2, 3):
                for k in range(KT):
                    nc.sync.dma_start(
                        out=xt[k][:, ch * NC512:(ch + 1) * NC512],
                        in_=xT[k * P:(k + 1) * P, ch * NC512:(ch + 1) * NC512])
            for k in range(KT):
                nc.sync.dma_start(out=wt[k][:, 2 * CH:3 * CH],
                                  in_=wqkv[k * P:(k + 1) * P, 2 * CH:3 * CH])
            for c in range(PAIRS):
                nc.sync.dma_start(out=wout_sb[c],
                                  in_=wout[c * P:(c + 1) * P, :])

            # ---------------- projection helpers ----------------
            def emit_v_proj(j, half, width):
                ps = psP.tile([P, width], f32, tag="pp",
                              name=f"pv{j}_{half}")
                c0 = half * width
                for k in range(KT):
                    nc.tensor.matmul(
                        ps,
                        lhsT=xt[k][:, j * P:(j + 1) * P],
                        rhs=wt[k][:, 2 * CH + c0:2 * CH + c0 + width],
                        start=(k == 0), stop=(k == KT - 1))
                nc.vector.tensor_copy(vnat[j][:, c0:c0 + width], ps)

            def emit_qk_chain(pr, sec, ch, width):
                dst = (qT if sec == 0 else kT)[pr % 2]
                ps = psP.tile([P, width], f32, tag="pp",
                              name=f"qk{pr}_{sec}_{ch}")
                i0 = ch * width
                for k in range(KT):
                    nc.tensor.matmul(
                        ps,
                        lhsT=wt[k][:, sec * CH + pr * P:sec * CH + (pr + 1) * P],
                        rhs=xt[k][:, i0:i0 + width],
                        start=(k == 0), stop=(k == KT - 1))
                nc.vector.tensor_copy(dst[:, i0:i0 + width], ps)

            def emit_swap(pr, idx):
                """Build the partition-swapped q/k copies for pair pr:
                qX/kX rows 0-63 = head B, rows 64-127 = head A."""
                b = pr % 2
                src, dst = (qT, qX) if idx // 2 == 0 else (kT, kX)
                if idx % 2 == 0:
                    nc.vector.tensor_copy(dst[b][0:DH, :], src[b][DH:P, :])
                else:
                    nc.vector.tensor_copy(dst[b][DH:P, :], src[b][0:DH, :])

            def emit_out_chain(o, ich):
                pool = psP if (o * 4 + ich) % 2 == 0 else psY
                tag = "pp" if pool is psP else "yy"
                ps = pool.tile([P, NC512], f32, tag=tag, name=f"po{o}_{ich}")
                for c in range(PAIRS):
                    nc.tensor.matmul(
                        ps,
                        lhsT=wout_sb[c][:, o * P:(o + 1) * P],
                        rhs=y_sb[c][:, ich * NC512:(ich + 1) * NC512],
                        start=(c == 0), stop=(c == PAIRS - 1))
                osb = p_osb.tile([P, NC512], f16, tag="osb", name="osb")
                nc.vector.tensor_scalar_mul(osb, ps, 1.0 / VPS)
                nc.sync.dma_start(
                    out=outT[o * P:(o + 1) * P,
                             ich * NC512:(ich + 1) * NC512],
                    in_=osb)

            # ---------------- prelude ----------------
            emit_qk_chain(0, 1, 0, 512)
            for ch in range(4):
                emit_qk_chain(0, 0, ch, 512)
            for ch in range(1, 4):
                emit_qk_chain(0, 1, ch, 512)
            for idx in range(4):
                emit_swap(0, idx)
            for j in range(JB):
                emit_v_proj(j, 0, 512)

            def filler_list(pr):
                work = []
                if pr == 0:
                    for j in range(JB, NT):
                        for h in range(2):
                            work.append(("v", j, h))
                if pr + 1 < PAIRS:
                    for ch in range(8):
                        work.append(("k", pr + 1, ch))
                    for ch in range(8):
                        work.append(("q", pr + 1, ch))
                    for idx in range(4):
                        work.append(("s", pr + 1, idx))
                return work

            def emit_filler(item):
                kind, a, b_ = item
                if kind == "v":
                    emit_v_proj(a, b_, 256)
                elif kind == "k":
                    emit_qk_chain(a, 1, b_, 256)
                elif kind == "q":
                    emit_qk_chain(a, 0, b_, 256)
                else:
                    emit_swap(a, b_)

            # ---------------- attention ----------------
            ats = {}    # (pr, ho, j, half) -> at tile (A: half=0 full width)
            vp = {}     # (pr, ho, j) -> fp8 vp tile (v * 4096/den)
            y_queue = []

            def make_y_chunk(pr, blk, ic):
                jlist = list(range(blk * JB, (blk + 1) * JB))

                def emit():
                    hf, c2 = divmod(ic, 2)
                    yb = psY.tile([P, NC512], f32, tag="yy",
                                  name=f"yb{pr}_{blk}_{ic}")
                    for idx, jj in enumerate(jlist):
                        for ho in (0, DH):
                            if ho == 0:
                                rhs = ats[(pr, 0, jj, 0)][:, ic * NC512:
                                                          (ic + 1) * NC512]
                            else:
                                rhs = ats[(pr, DH, jj, hf)][:, c2 * NC512:
                                                            (c2 + 1) * NC512]
                            nc.tensor.matmul(
                                yb[ho:ho + DH, :],
                                lhsT=vp[(pr, ho, jj)],
                                rhs=rhs,
                                start=(idx == 0),
                                stop=(idx == JB - 1),
                                tile_position=(0, ho))
                    i0 = hf * S_W + c2 * NC512
                    if blk == 0:
                        nc.vector.tensor_copy(y_sb[pr][:, i0:i0 + NC512], yb)
                    else:
                        nc.vector.tensor_add(
                            y_sb[pr][:, i0:i0 + NC512],
                            y_sb[pr][:, i0:i0 + NC512], yb)
                    if ic == 3:
                        for jj in jlist:
                            ats.pop((pr, 0, jj, 0), None)
                            ats.pop((pr, DH, jj, 0), None)
                            ats.pop((pr, DH, jj, 1), None)
                            vp.pop((pr, 0, jj), None)
                            vp.pop((pr, DH, jj), None)
                return emit

            def fill_scores(pr, j, ho, s, width, ioff):
                """Packed score fill: even 512-chunks on PE rows 0-63, odd
                chunks on rows 64-127 via the swapped copies."""
                b = pr % 2
                js = slice(j * P, (j + 1) * P)
                nch = width // NC512
                for c in range(nch):
                    i0 = ioff + c * NC512
                    if (c % 2 == 0) == (ho == 0):
                        # head's rows live at partitions 0-63 in this source
                        kt_, qt_ = (kT[b], qT[b]) if ho == 0 else (kX[b], qX[b])
                        nc.tensor.matmul(
                            s[:, c * NC512:(c + 1) * NC512],
                            lhsT=kt_[0:DH, js],
                            rhs=qt_[0:DH, i0:i0 + NC512],
                            start=True, stop=True,
                            tile_position=(0, 0))
                    else:
                        kt_, qt_ = (kX[b], qX[b]) if ho == 0 else (kT[b], qT[b])
                        nc.tensor.matmul(
                            s[:, c * NC512:(c + 1) * NC512],
                            lhsT=kt_[DH:P, js],
                            rhs=qt_[DH:P, i0:i0 + NC512],
                            start=True, stop=True,
                            tile_position=(DH, 0))

            for pr in range(PAIRS):
                filler = filler_list(pr)
                fi = 0
                for j in range(NT):
                    quota = -(-(len(filler) - fi) // (NT - j))  # ceil
                    if fi < len(filler) and quota > 0:
                        emit_filler(filler[fi])
                        fi += 1
                        quota -= 1
                    dens = {}
                    # head A: one FD=2048 exp from the 4-bank slot
                    sA = psSA.tile([P, N], f32, tag="sa", name=f"sa{pr}_{j}")
                    fill_scores(pr, j, 0, sA, N, 0)
                    atA = p_atA.tile([P, N], f16, tag="atA", name="atA")
                    denA = p_den.tile([P, 1], f32, tag="den", name="denA")
                    nc.scalar.activation(atA, sA, EXP, bias=neg1,
                                         scale=SCALE, accum_out=denA)
                    ats[(pr, 0, j, 0)] = atA
                    # pending y-chunk between the A and B groups
                    if y_queue:
                        y_queue.pop(0)()
                    # head B: two FD=1024 exps from the 2-bank slot
                    for hf in range(2):
                        sB = psSB.tile([P, S_W], f32, tag="sb",
                                       name=f"sb{pr}_{j}_{hf}")
                        fill_scores(pr, j, DH, sB, S_W, hf * S_W)
                        atB = p_atB.tile([P, S_W], f16, tag="atB", name="atB")
                        den = p_den.tile([P, 1], f32, tag="den",
                                         name=f"denB{hf}")
                        nc.scalar.activation(atB, sB, EXP, bias=neg1,
                                             scale=SCALE, accum_out=den)
                        ats[(pr, DH, j, hf)] = atB
                        dens[hf] = den
                    # rest of this step's filler quota
                    while fi < len(filler) and quota > 0:
                        emit_filler(filler[fi])
                        fi += 1
                        quota -= 1
                    # denominators -> reciprocal -> fp8 vp (scaled by VPS)
                    recA = p_den.tile([P, 1], f32, tag="den", name="recA")
                    nc.vector.reciprocal(recA, denA)
                    vpA = p_vp.tile([P, DH], f16, tag="vp", name="vpA")
                    nc.vector.tensor_scalar(
                        vpA, vnat[j][:, pr * 2 * DH:pr * 2 * DH + DH],
                        recA, VPS, MUL, MUL)
                    vp[(pr, 0, j)] = vpA
                    dtot = p_den.tile([P, 1], f32, tag="den", name="dtot")
                    nc.vector.tensor_add(dtot, dens[0], dens[1])
                    recB = p_den.tile([P, 1], f32, tag="den", name="recB")
                    nc.vector.reciprocal(recB, dtot)
                    vpB = p_vp.tile([P, DH], f16, tag="vp", name="vpB")
                    c0 = pr * 2 * DH + DH
                    nc.vector.tensor_scalar(
                        vpB, vnat[j][:, c0:c0 + DH], recB, VPS, MUL, MUL)
                    vp[(pr, DH, j)] = vpB
                    if j % JB == JB - 1:
                        blk = j // JB
                        for ic in range(4):
                            y_queue.append(make_y_chunk(pr, blk, ic))
                while fi < len(filler):
                    emit_filler(filler[fi])
                    fi += 1

            # drain remaining y chunks (last block of last pair)
            while y_queue:
                y_queue.pop(0)()

            # ---------------- output projection (tail) ----------------
            for o in range(OT):
                for ich in range(N // NC512):
                    emit_out_chain(o, ich)
    return nc


def _shard_inputs(x, w_qkv, w_out):
    """Build per-core input maps: core c -> (batch c//2, head-half c%2)."""
    in_maps = []
    for c in range(N_CORES):
        b, hh = c // 2, c % 2
        cols = slice(hh * CH, (hh + 1) * CH)
        xTc = np.ascontiguousarray(np.asarray(x[b]).T, dtype=np.float16)
        wq = w_qkv[:, 0 * F:1 * F][:, cols]
        wk = w_qkv[:, 1 * F:2 * F][:, cols]
        wv = w_qkv[:, 2 * F:3 * F][:, cols]
        wqkv_c = np.ascontiguousarray(
            np.concatenate([wq, wk, wv], axis=1), dtype=np.float16)
        wout_c = np.ascontiguousarray(w_out[cols, :], dtype=np.float16)
        in_maps.append({"xT": xTc, "wqkv": wqkv_c, "wout": wout_c})
    return in_maps


def _gather_outputs(results, b_out):
    out = np.empty((B, N, OUT), np.float32)
    bias = np.asarray(b_out, dtype=np.float32)
    for b in range(B):
        acc = (results[2 * b]["outT"].astype(np.float32)
               + results[2 * b + 1]["outT"].astype(np.float32))  # [OUT, N]
        out[b] = acc.T + bias
    return out


# Test instrumentation (harness just calls kernel(); these stay default).
_TRACE = False
_LAST_RESULT = None


def kernel(x, w_qkv, w_out, b_out):
    global _LAST_RESULT
    # The bass->PJRT path needs the axon trn2 devices visible to jax.
    if os.environ.get("JAX_PLATFORMS") not in (None, "", "axon"):
        os.environ.pop("JAX_PLATFORMS", None)
    from concourse.bass_utils import run_bass_kernel_spmd

    nc = _build_nc()
    if not nc.is_finalized():
        nc.finalize()  # runs Bacc legalization (wait splitting, reg alloc)
    in_maps = _shard_inputs(np.asarray(x), np.asarray(w_qkv),
                            np.asarray(w_out))
    res = run_bass_kernel_spmd(nc, in_maps, list(range(N_CORES)),
                               trace=_TRACE)
    _LAST_RESULT = res
    return _gather_outputs(res.results, np.asarray(b_out))


# ---------------------------------------------------------------------------
# Numpy emulation of the per-core device program (for host-logic testing;
# not used by kernel()).
def _emulate_core(m):
    xT, wqkv, wout = m["xT"], m["wqkv"], m["wout"]
    qT = (wqkv[:, 0:CH].T @ xT)          # [CH, N]
    kTm = (wqkv[:, CH:2 * CH].T @ xT)    # [CH, N]
    v = xT.T @ wqkv[:, 2 * CH:3 * CH]    # [N, CH]
    y = np.empty((CH, N), np.float32)
    for h in range(HH):
        qh = qT[h * DH:(h + 1) * DH, :]      # [DH, N(i)]
        kh = kTm[h * DH:(h + 1) * DH, :]     # [DH, N(j)]
        sT = kh.T @ qh                       # [j, i]
        e = np.exp(sT * SCALE - 1.0)
        den = e.sum(axis=1, keepdims=True)   # over queries i, per key j
        vpm = v[:, h * DH:(h + 1) * DH] * (VPS / den)
        y[h * DH:(h + 1) * DH, :] = (vpm.T @ e) / VPS  # [DH, i]
    outT_acc = wout.T @ y                    # [OUT, N]
    return outT_acc.astype(np.float16)


def _kernel_emulated(x, w_qkv, w_out, b_out):
    in_maps = _shard_inputs(np.asarray(x), np.asarray(w_qkv),
                            np.asarray(w_out))
    results = [{"outT": _emulate_core(m)} for m in in_maps]
    return _gather_outputs(results, np.asarray(b_out))


# revision 16
# speedup vs baseline: 1.0598x; 1.0007x over previous
"""Trainium2 Bass kernel for MHA with query-axis softmax (nn_MHA_2568390443327).

Reference computation (B=4, N=2048, DIM=1024, 16 heads x 64):
    qkv = x @ w_qkv ; q,k,v = split(qkv)
    scores = (q @ k^T) * scale            # [b,h,i(query),j(key)]
    attn = softmax(scores, axis=QUERY)    # normalized over i, per key j
    y = attn @ v ; out = y @ w_out + b_out

Sharding (8 cores): batch (4) x head-half (2). Each core gets its batch's
x (pre-transposed), the qkv weight columns and w_out rows for its 8 heads,
and produces a partial [DIM, N] fp16 output (transposed). Host sums the two
head-half partials per batch, adds the bias, and transposes back.

v4 design: the exp of all N^2 scores on ScalarE (1 elem/lane/cycle) is the
hard floor, so everything serves ACT saturation while shrinking PE work:
  - Mixed-FD exps: head A of each pair gets one FD=2048 activation from a
    4-bank PSUM slot; head B gets two FD=1024 from a 2-bank slot. Single
    buffering per slot still ping-pongs (A's refill runs under B's exps
    and vice versa). PSUM: 4 (S_A) + 2 (S_B) + 1 (y) + 1 (proj) = 8 banks.
  - Score fills tile-pack 2x using swapped q/k copies (qx/kx hold [B;A]
    so each head has its rows on both partition halves): even i-chunks run
    on PE rows 0-63, odd on 64-127, concurrently.
  - exp outputs (at) are fp8e4: halves SBUF and keeps both y-matmul
    operands fp8 (vp too, scaled by 4096 via the reciprocal path; undone
    in the final output copy). exp runs with bias=-1 as an fp8-overflow
    guard, which cancels exactly in the den normalization.
  - y accumulates over j in 4-j-tile PSUM blocks, emitted one block late,
    one i-chunk per j-step; qkv/out projection chains (256-free) thread
    into remaining PE slack.
"""

import os
import numpy as np

# ---------------------------------------------------------------------------
# Problem constants (hardcoded; kernel.py must be self-contained).
B = 4
N = 2048          # sequence length
F = 1024          # model dim (contraction for qkv proj)
HEADS_TOT = 16
DH = 64           # head dim
HH = 8            # heads per core (head-half)
CH = HH * DH      # 512: per-core hidden
OUT = 1024        # output dim
SCALE = 0.125     # 1/sqrt(64)
N_CORES = 8

P = 128           # partitions
NC512 = 512       # matmul free-dim chunk (one PSUM bank of fp32)
S_W = 1024        # head-B score tile width (half the i range)
JB = 4            # j-tiles per y accumulation block
VPS = 1.0         # vp scale (fp8 experiment disabled)


def _build_nc():
    import concourse.bass as bass  # noqa: F401
    import concourse.mybir as mybir
    from concourse import bacc
    from concourse.tile import TileContext

    f32 = mybir.dt.float32
    f16 = mybir.dt.float16
    f8 = mybir.dt.float8e4
    EXP = mybir.ActivationFunctionType.Exp
    MUL = mybir.AluOpType.mult

    nc = bacc.Bacc(None, target_bir_lowering=False)

    xT = nc.declare_dram_parameter("xT", [F, N], f16, isOutput=False)
    wqkv = nc.declare_dram_parameter("wqkv", [F, 3 * CH], f16, isOutput=False)
    wout = nc.declare_dram_parameter("wout", [CH, OUT], f16, isOutput=False)
    outT = nc.declare_dram_parameter("outT", [OUT, N], f16, isOutput=True)

    KT = F // P            # 8 k-tiles for qkv projection contraction
    NT = N // P            # 16 j-tiles (key blocks)
    PAIRS = 4              # head pairs per core (2 heads each)
    OT = OUT // P          # 8 output row tiles

    with TileContext(nc) as tc:
        with (
            tc.tile_pool(name="p_x", bufs=1) as p_x,
            tc.tile_pool(name="p_w", bufs=1) as p_w,
            tc.tile_pool(name="p_wout", bufs=1) as p_wout,
            tc.tile_pool(name="p_v", bufs=1) as p_v,
            tc.tile_pool(name="p_qkT", bufs=1) as p_qkT,
            tc.tile_pool(name="p_ysb", bufs=1) as p_ysb,
            tc.tile_pool(name="p_atA", bufs=9) as p_atA,
            tc.tile_pool(name="p_atB", bufs=16) as p_atB,
            tc.tile_pool(name="p_den", bufs=64) as p_den,
            tc.tile_pool(name="p_vp", bufs=20) as p_vp,
            tc.tile_pool(name="p_osb", bufs=4) as p_osb,
            tc.tile_pool(name="psSA", bufs=1, space="PSUM") as psSA,
            tc.tile_pool(name="psSB", bufs=1, space="PSUM") as psSB,
            tc.tile_pool(name="psY", bufs=1, space="PSUM") as psY,
            tc.tile_pool(name="psP", bufs=1, space="PSUM") as psP,
        ):
            # ---------------- persistent SBUF tiles ----------------
            xt = [p_x.tile([P, N], f16, tag=f"x{k}", name=f"x{k}")
                  for k in range(KT)]
            wt = [p_w.tile([P, 3 * CH], f16, tag=f"w{k}", name=f"w{k}")
                  for k in range(KT)]
            wout_sb = [p_wout.tile([P, OUT], f16, tag=f"wo{c}", name=f"wo{c}")
                       for c in range(PAIRS)]
            vnat = [p_v.tile([P, CH], f16, tag=f"v{j}", name=f"v{j}")
                    for j in range(NT)]
            # q/k of current+next pair, plus partition-swapped copies [B;A]
            qT = [p_qkT.tile([P, N], f16, tag=f"qT{i}", name=f"qT{i}")
                  for i in range(2)]
            kT = [p_qkT.tile([P, N], f16, tag=f"kT{i}", name=f"kT{i}")
                  for i in range(2)]
            qX = [p_qkT.tile([P, N], f16, tag=f"qX{i}", name=f"qX{i}")
                  for i in range(2)]
            kX = [p_qkT.tile([P, N], f16, tag=f"kX{i}", name=f"kX{i}")
                  for i in range(2)]
            # per-pair fp16 y accumulators (hold 4096*y; all pairs kept)
            y_sb = [p_ysb.tile([P, N], f16, tag=f"y{p_}", name=f"y{p_}")
                    for p_ in range(PAIRS)]

            neg1 = p_osb.tile([P, 1], f32, tag="neg1", name="neg1")
            nc.vector.memset(neg1, -1.0)

            # DMA in dependency order: pair-0 k/q weight sections and the
            # first x i-chunk land first so projection chains start early.
            for k in range(KT):
                nc.sync.dma_start(out=wt[k][:, 0:CH],
                                  in_=wqkv[k * P:(k + 1) * P, 0:CH])
            for k in range(KT):
                nc.sync.dma_start(out=xt[k][:, 0:NC512],
                                  in_=xT[k * P:(k + 1) * P, 0:NC512])
            for k in range(KT):
                nc.sync.dma_start(out=wt[k][:, CH:2 * CH],
                                  in_=wqkv[k * P:(k + 1) * P, CH:2 * CH])
            for ch in (1, 2, 3):
                for k in range(KT):
                    nc.sync.dma_start(
                        out=xt[k][:, ch * NC512:(ch + 1) * NC512],
                        in_=xT[k * P:(k + 1) * P, ch * NC512:(ch + 1) * NC512])
            for k in range(KT):
                nc.sync.dma_start(out=wt[k][:, 2 * CH:3 * CH],
                                  in_=wqkv[k * P:(k + 1) * P, 2 * CH:3 * CH])
            for c in range(PAIRS):
                nc.sync.dma_start(out=wout_sb[c],
                                  in_=wout[c * P:(c + 1) * P, :])

            # ---------------- projection helpers ----------------
            def emit_v_proj(j, half, width):
                ps = psP.tile([P, width], f32, tag="pp",
                              name=f"pv{j}_{half}")
                c0 = half * width
                for k in range(KT):
                    nc.tensor.matmul(
                        ps,
                        lhsT=xt[k][:, j * P:(j + 1) * P],
                        rhs=wt[k][:, 2 * CH + c0:2 * CH + c0 + width],
                        start=(k == 0), stop=(k == KT - 1))
                nc.vector.tensor_copy(vnat[j][:, c0:c0 + width], ps)

            def emit_qk_chain(pr, sec, ch, width):
                dst = (qT if sec == 0 else kT)[pr % 2]
                ps = psP.tile([P, width], f32, tag="pp",
                              name=f"qk{pr}_{sec}_{ch}")
                i0 = ch * width
                for k in range(KT):
                    nc.tensor.matmul(
                        ps,
                        lhsT=wt[k][:, sec * CH + pr * P:sec * CH + (pr + 1) * P],
                        rhs=xt[k][:, i0:i0 + width],
                        start=(k == 0), stop=(k == KT - 1))
                nc.vector.tensor_copy(dst[:, i0:i0 + width], ps)

            def emit_swap(pr, idx):
                """Build the partition-swapped q/k copies for pair pr:
                qX/kX rows 0-63 = head B, rows 64-127 = head A."""
                b = pr % 2
                src, dst = (qT, qX) if idx // 2 == 0 else (kT, kX)
                if idx % 2 == 0:
                    nc.vector.tensor_copy(dst[b][0:DH, :], src[b][DH:P, :])
                else:
                    nc.vector.tensor_copy(dst[b][DH:P, :], src[b][0:DH, :])

            def emit_out_chain(o, ich):
                pool = psP if (o * 4 + ich) % 2 == 0 else psY
                tag = "pp" if pool is psP else "yy"
                ps = pool.tile([P, NC512], f32, tag=tag, name=f"po{o}_{ich}")
                for c in range(PAIRS):
                    nc.tensor.matmul(
                        ps,
                        lhsT=wout_sb[c][:, o * P:(o + 1) * P],
                        rhs=y_sb[c][:, ich * NC512:(ich + 1) * NC512],
                        start=(c == 0), stop=(c == PAIRS - 1))
                osb = p_osb.tile([P, NC512], f16, tag="osb", name="osb")
                nc.vector.tensor_scalar_mul(osb, ps, 1.0 / VPS)
                nc.sync.dma_start(
                    out=outT[o * P:(o + 1) * P,
                             ich * NC512:(ich + 1) * NC512],
                    in_=osb)

            # ---------------- prelude ----------------
            emit_qk_chain(0, 0, 0, 512)
            emit_qk_chain(0, 1, 0, 512)
            for ch in range(1, 4):
                emit_qk_chain(0, 0, ch, 512)
            for ch in range(1, 4):
                emit_qk_chain(0, 1, ch, 512)
            for idx in range(4):
                emit_swap(0, idx)
            for j in range(JB):
                emit_v_proj(j, 0, 512)

            def filler_list(pr):
                work = []
                if pr == 0:
                    for j in range(JB, NT):
                        for h in range(2):
                            work.append(("v", j, h))
                if pr + 1 < PAIRS:
                    for ch in range(8):
                        work.append(("k", pr + 1, ch))
                    for ch in range(8):
                        work.append(("q", pr + 1, ch))
                    for idx in range(4):
                        work.append(("s", pr + 1, idx))
                return work

            def emit_filler(item):
                kind, a, b_ = item
                if kind == "v":
                    emit_v_proj(a, b_, 256)
                elif kind == "k":
                    emit_qk_chain(a, 1, b_, 256)
                elif kind == "q":
                    emit_qk_chain(a, 0, b_, 256)
                else:
                    emit_swap(a, b_)

            # ---------------- attention ----------------
            ats = {}    # (pr, ho, j, half) -> at tile (A: half=0 full width)
            vp = {}     # (pr, ho, j) -> fp8 vp tile (v * 4096/den)
            y_queue = []

            def make_y_chunk(pr, blk, ic):
                jlist = list(range(blk * JB, (blk + 1) * JB))

                def emit():
                    hf, c2 = divmod(ic, 2)
                    yb = psY.tile([P, NC512], f32, tag="yy",
                                  name=f"yb{pr}_{blk}_{ic}")
                    for idx, jj in enumerate(jlist):
                        for ho in (0, DH):
                            if ho == 0:
                                rhs = ats[(pr, 0, jj, 0)][:, ic * NC512:
                                                          (ic + 1) * NC512]
                            else:
                                rhs = ats[(pr, DH, jj, hf)][:, c2 * NC512:
                                                            (c2 + 1) * NC512]
                            nc.tensor.matmul(
                                yb[ho:ho + DH, :],
                                lhsT=vp[(pr, ho, jj)],
                                rhs=rhs,
                                start=(idx == 0),
                                stop=(idx == JB - 1),
                                tile_position=(0, ho))
                    i0 = hf * S_W + c2 * NC512
                    if blk == 0:
                        nc.vector.tensor_copy(y_sb[pr][:, i0:i0 + NC512], yb)
                    else:
                        nc.vector.tensor_add(
                            y_sb[pr][:, i0:i0 + NC512],
                            y_sb[pr][:, i0:i0 + NC512], yb)
                    if ic == 3:
                        for jj in jlist:
                            ats.pop((pr, 0, jj, 0), None)
                            ats.pop((pr, DH, jj, 0), None)
                            ats.pop((pr, DH, jj, 1), None)
                            vp.pop((pr, 0, jj), None)
                            vp.pop((pr, DH, jj), None)
                return emit

            def fill_scores(pr, j, ho, s, width, ioff, packed=True):
                """Packed score fill: even 512-chunks on PE rows 0-63, odd
                chunks on rows 64-127 via the swapped copies. Unpacked mode
                skips the swapped tiles (used before they exist)."""
                b = pr % 2
                js = slice(j * P, (j + 1) * P)
                nch = width // NC512
                if not packed:
                    for c in range(nch):
                        i0 = ioff + c * NC512
                        nc.tensor.matmul(
                            s[:, c * NC512:(c + 1) * NC512],
                            lhsT=kT[b][ho:ho + DH, js],
                            rhs=qT[b][ho:ho + DH, i0:i0 + NC512],
                            start=True, stop=True,
                            tile_position=(ho, 0))
                    return
                for c in range(nch):
                    i0 = ioff + c * NC512
                    if (c % 2 == 0) == (ho == 0):
                        # head's rows live at partitions 0-63 in this source
                        kt_, qt_ = (kT[b], qT[b]) if ho == 0 else (kX[b], qX[b])
                        nc.tensor.matmul(
                            s[:, c * NC512:(c + 1) * NC512],
                            lhsT=kt_[0:DH, js],
                            rhs=qt_[0:DH, i0:i0 + NC512],
                            start=True, stop=True,
                            tile_position=(0, 0))
                    else:
                        kt_, qt_ = (kX[b], qX[b]) if ho == 0 else (kT[b], qT[b])
                        nc.tensor.matmul(
                            s[:, c * NC512:(c + 1) * NC512],
                            lhsT=kt_[DH:P, js],
                            rhs=qt_[DH:P, i0:i0 + NC512],
                            start=True, stop=True,
                            tile_position=(DH, 0))

            for pr in range(PAIRS):
                filler = filler_list(pr)
                fi = 0
                for j in range(NT):
                    quota = -(-(len(filler) - fi) // (NT - j))  # ceil
                    if fi < len(filler) and quota > 0:
                        emit_filler(filler[fi])
                        fi += 1
                        quota -= 1
                    dens = {}
                    # head A: one FD=2048 exp from the 4-bank slot
                    sA = psSA.tile([P, N], f32, tag="sa", name=f"sa{pr}_{j}")
                    fill_scores(pr, j, 0, sA, N, 0,
                                packed=(pr > 0 or j >= 6))
                    atA = p_atA.tile([P, N], f16, tag="atA", name="atA")
                    denA = p_den.tile([P, 1], f32, tag="den", name="denA")
                    nc.scalar.activation(atA, sA, EXP, bias=neg1,
                                         scale=SCALE, accum_out=denA)
                    ats[(pr, 0, j, 0)] = atA
                    # pending y-chunk between the A and B groups
                    if y_queue:
                        y_queue.pop(0)()
                    # head B: two FD=1024 exps from the 2-bank slot
                    for hf in range(2):
                        sB = psSB.tile([P, S_W], f32, tag="sb",
                                       name=f"sb{pr}_{j}_{hf}")
                        fill_scores(pr, j, DH, sB, S_W, hf * S_W,
                                    packed=(pr > 0 or j >= 6))
                        atB = p_atB.tile([P, S_W], f16, tag="atB", name="atB")
                        den = p_den.tile([P, 1], f32, tag="den",
                                         name=f"denB{hf}")
                        nc.scalar.activation(atB, sB, EXP, bias=neg1,
                                             scale=SCALE, accum_out=den)
                        ats[(pr, DH, j, hf)] = atB
                        dens[hf] = den
                    # rest of this step's filler quota
                    while fi < len(filler) and quota > 0:
                        emit_filler(filler[fi])
                        fi += 1
                        quota -= 1
                    # denominators -> reciprocal -> fp8 vp (scaled by VPS)
                    recA = p_den.tile([P, 1], f32, tag="den", name="recA")
                    nc.vector.reciprocal(recA, denA)
                    vpA = p_vp.tile([P, DH], f16, tag="vp", name="vpA")
                    nc.vector.tensor_scalar(
                        vpA, vnat[j][:, pr * 2 * DH:pr * 2 * DH + DH],
                        recA, VPS, MUL, MUL)
                    vp[(pr, 0, j)] = vpA
                    dtot = p_den.tile([P, 1], f32, tag="den", name="dtot")
                    nc.vector.tensor_add(dtot, dens[0], dens[1])
                    recB = p_den.tile([P, 1], f32, tag="den", name="recB")
                    nc.vector.reciprocal(recB, dtot)
                    vpB = p_vp.tile([P, DH], f16, tag="vp", name="vpB")
                    c0 = pr * 2 * DH + DH
                    nc.vector.tensor_scalar(
                        vpB, vnat[j][:, c0:c0 + DH], recB, VPS, MUL, MUL)
                    vp[(pr, DH, j)] = vpB
                    if j % JB == JB - 1:
                        blk = j // JB
                        for ic in range(4):
                            y_queue.append(make_y_chunk(pr, blk, ic))
                while fi < len(filler):
                    emit_filler(filler[fi])
                    fi += 1

            # drain remaining y chunks (last block of last pair)
            while y_queue:
                y_queue.pop(0)()

            # ---------------- output projection (tail) ----------------
            for o in range(OT):
                for ich in range(N // NC512):
                    emit_out_chain(o, ich)
    return nc


def _shard_inputs(x, w_qkv, w_out):
    """Build per-core input maps: core c -> (batch c//2, head-half c%2)."""
    in_maps = []
    for c in range(N_CORES):
        b, hh = c // 2, c % 2
        cols = slice(hh * CH, (hh + 1) * CH)
        xTc = np.ascontiguousarray(np.asarray(x[b]).T, dtype=np.float16)
        wq = w_qkv[:, 0 * F:1 * F][:, cols]
        wk = w_qkv[:, 1 * F:2 * F][:, cols]
        wv = w_qkv[:, 2 * F:3 * F][:, cols]
        wqkv_c = np.ascontiguousarray(
            np.concatenate([wq, wk, wv], axis=1), dtype=np.float16)
        wout_c = np.ascontiguousarray(w_out[cols, :], dtype=np.float16)
        in_maps.append({"xT": xTc, "wqkv": wqkv_c, "wout": wout_c})
    return in_maps


def _gather_outputs(results, b_out):
    out = np.empty((B, N, OUT), np.float32)
    bias = np.asarray(b_out, dtype=np.float32)
    for b in range(B):
        acc = (results[2 * b]["outT"].astype(np.float32)
               + results[2 * b + 1]["outT"].astype(np.float32))  # [OUT, N]
        out[b] = acc.T + bias
    return out


# Test instrumentation (harness just calls kernel(); these stay default).
_TRACE = False
_LAST_RESULT = None


def kernel(x, w_qkv, w_out, b_out):
    global _LAST_RESULT
    # The bass->PJRT path needs the axon trn2 devices visible to jax.
    if os.environ.get("JAX_PLATFORMS") not in (None, "", "axon"):
        os.environ.pop("JAX_PLATFORMS", None)
    from concourse.bass_utils import run_bass_kernel_spmd

    nc = _build_nc()
    if not nc.is_finalized():
        nc.finalize()  # runs Bacc legalization (wait splitting, reg alloc)
    in_maps = _shard_inputs(np.asarray(x), np.asarray(w_qkv),
                            np.asarray(w_out))
    res = run_bass_kernel_spmd(nc, in_maps, list(range(N_CORES)),
                               trace=_TRACE)
    _LAST_RESULT = res
    return _gather_outputs(res.results, np.asarray(b_out))


# ---------------------------------------------------------------------------
# Numpy emulation of the per-core device program (for host-logic testing;
# not used by kernel()).
def _emulate_core(m):
    xT, wqkv, wout = m["xT"], m["wqkv"], m["wout"]
    qT = (wqkv[:, 0:CH].T @ xT)          # [CH, N]
    kTm = (wqkv[:, CH:2 * CH].T @ xT)    # [CH, N]
    v = xT.T @ wqkv[:, 2 * CH:3 * CH]    # [N, CH]
    y = np.empty((CH, N), np.float32)
    for h in range(HH):
        qh = qT[h * DH:(h + 1) * DH, :]      # [DH, N(i)]
        kh = kTm[h * DH:(h + 1) * DH, :]     # [DH, N(j)]
        sT = kh.T @ qh                       # [j, i]
        e = np.exp(sT * SCALE - 1.0)
        den = e.sum(axis=1, keepdims=True)   # over queries i, per key j
        vpm = v[:, h * DH:(h + 1) * DH] * (VPS / den)
        y[h * DH:(h + 1) * DH, :] = (vpm.T @ e) / VPS  # [DH, i]
    outT_acc = wout.T @ y                    # [OUT, N]
    return outT_acc.astype(np.float16)


def _kernel_emulated(x, w_qkv, w_out, b_out):
    in_maps = _shard_inputs(np.asarray(x), np.asarray(w_qkv),
                            np.asarray(w_out))
    results = [{"outT": _emulate_core(m)} for m in in_maps]
    return _gather_outputs(results, np.asarray(b_out))
